# revision 1
# baseline (speedup 1.0000x reference)
"""Trainium2 Bass kernel for nn_FFN_Shared_Plus_TaskLoRA (moe_routing).

Computation (per token x in R^768):
    y   = gelu_tanh(x @ (W1+dW1)^T) @ (W2+dW2)^T          (biases are all zero)
    g   = top2-softmax(x @ Wg^T + 0.1*noise)              (dense [E=8] weights)
    moe = sum_e g_e * gelu_tanh(x @ We1[e]^T) @ We2[e]^T
    out = y + moe
Sharding: data-parallel over tokens, 2048/core, weights replicated.

v3 layout (per core, per 2048-token iteration):
  - G phase (once per iteration): gate logits for all 2048 tokens in
    fp32r (from an fp32 copy of x — bf16 is too coarse for the top-2
    ranking), PE-transpose to token-major, one batched top-2 softmax
    (one Exp per iteration -> only 2 ACT table reloads per iteration),
    combine weights transposed back to expert-major wT [8, 2048] bf16.
    The PE transposes that depend on the DVE softmax chain are emitted
    after chunk-0 B1 so the PE never waits on DVE/ACT.
  - 4 chunks of 512 tokens: B1 hT = gelu(W1 @ xT) (24 bf16 tiles),
    expert hcat/gating, then B2 with per-m-tile serial k-loops
    accumulating moe + W2 @ hT.  W1, W2 both SBUF-resident in bf16.
  - All PSUM->SBUF copies on DVE (ACT runs gelu only; no table thrash).
"""
import os
import sys

sys.path.insert(0, '/opt/trn_rl_repo')
os.environ.setdefault('BASS_NEVER_TRACE', '1')

from contextlib import ExitStack

import numpy as np
import ml_dtypes

import concourse.bacc as bacc
import concourse.tile as tile
from concourse import mybir
from concourse.masks import make_identity

F32 = mybir.dt.float32
F32R = mybir.dt.float32r
BF16 = mybir.dt.bfloat16
NP_BF16 = ml_dtypes.bfloat16
AF = mybir.ActivationFunctionType
ALU = mybir.AluOpType
AX = mybir.AxisListType

B, N, D = 4, 4096, 768
MID = 4 * D              # 3072
E, INNER = 8, 24
EI = E * INNER           # 192
NOISE_STD = 0.1
NCORES = 8
TPC = (B * N) // NCORES  # 2048 tokens per core
TC = 512                 # chunk of tokens
NCH = TPC // TC          # 4 chunks
KD = D // 128            # 6  k-tiles of the D contraction
KM = MID // 128          # 24 k-tiles of the MID contraction
MD = D // 128            # 6  m-tiles of the D output
NTT = TPC // 128         # 16 token sub-tiles per iteration


def build_core_program(loop_reps=None, phases='full', b2mode='rot3'):
    """Build the per-core Bass program. If loop_reps is given, the body is
    wrapped in a For_i (timing builds).

    phases: 'full' | 'b1' (loads+B1 only) | 'b1b2' (no gate/experts)
    b2mode: 'rot3' (two 3-m-tile passes, bank rotation) | 'serial'
            (per-m-tile 26-mm runs) | 'split' (two 13-accum halves + DVE add)
    """
    nc = bacc.Bacc('TRN2', target_bir_lowering=False, debug=False)

    xT_h = nc.dram_tensor('xT', [NCH, KD, 128, TC], BF16,
                          kind='ExternalInput').ap()
    xgT_h = nc.dram_tensor('xgT', [NCH, KD, 128, TC], F32R,
                           kind='ExternalInput').ap()
    noise_h = nc.dram_tensor('noise', [TPC, E], F32, kind='ExternalInput').ap()
    w1_h = nc.dram_tensor('w1T', [KD, 128, MID], BF16,
                          kind='ExternalInput').ap()
    w2_h = nc.dram_tensor('w2T', [KM, 128, D], BF16, kind='ExternalInput').ap()
    wg_h = nc.dram_tensor('wgT', [D, E], F32R, kind='ExternalInput').ap()
    we1_h = nc.dram_tensor('we1T', [D, EI], BF16, kind='ExternalInput').ap()
    we2_h = nc.dram_tensor('we2T', [EI, D], BF16, kind='ExternalInput').ap()
    bb_h = nc.dram_tensor('bb', [E, EI], BF16, kind='ExternalInput').ap()
    yT_h = nc.dram_tensor('yT', [D, TPC], F32, kind='ExternalOutput').ap()

    with tile.TileContext(nc) as tc:
        with ExitStack() as ctx:
            const = ctx.enter_context(tc.tile_pool(name='const', bufs=1))
            xp = ctx.enter_context(tc.tile_pool(name='xp', bufs=2))
            htp = ctx.enter_context(tc.tile_pool(name='htp', bufs=KM))
            gp = ctx.enter_context(tc.tile_pool(name='gp', bufs=2))
            ep = ctx.enter_context(tc.tile_pool(name='ep', bufs=1))
            op = ctx.enter_context(tc.tile_pool(name='op', bufs=3))
            nb_y = 4 if b2mode == 'split' else 3
            nb_a = 2 if b2mode == 'split' else 3
            ps_y = ctx.enter_context(tc.tile_pool(name='ps_y', bufs=nb_y, space='PSUM'))
            ps_h = ctx.enter_context(tc.tile_pool(name='ps_h', bufs=2, space='PSUM'))
            ps_a = ctx.enter_context(tc.tile_pool(name='ps_a', bufs=nb_a, space='PSUM'))

            ident = const.tile([128, 128], F32, tag='ident')
            make_identity(nc, ident[:])

            wg_t = const.tile([128, KD * E], F32R, tag='wg')
            nc.sync.dma_start(wg_t[:].rearrange('p (k e) -> p k e', k=KD),
                              wg_h.rearrange('(k p) e -> p k e', p=128))

            # ---- resident weights ----
            w1_t = []
            for k in range(KD):
                t = const.tile([128, MID], BF16, tag=f'w1_{k}')
                nc.sync.dma_start(t[:], w1_h[k])
                w1_t.append(t)
            w2_t = []
            for km in range(KM):
                t = const.tile([128, D], BF16, tag=f'w2_{km}')
                nc.sync.dma_start(t[:], w2_h[km])
                w2_t.append(t)
            we1_t = const.tile([128, KD * EI], BF16, tag='we1')
            nc.sync.dma_start(we1_t[:].rearrange('p (k i) -> p k i', k=KD),
                              we1_h.rearrange('(k p) i -> p k i', p=128))
            we2_t = const.tile([96, 2 * D], BF16, tag='we2')
            nc.sync.dma_start(we2_t[:].rearrange('p (k d) -> p k d', k=2),
                              we2_h.rearrange('(k p) d -> p k d', p=96))
            bb_t = const.tile([E, EI], BF16, tag='bb')
            nc.sync.dma_start(bb_t[:], bb_h[:])

            def emit_b2(c, ht, hs, it):
                """yT = [moe +] W2eff @ hT for one chunk; hs=None skips moe."""
                c0 = c * TC

                def moe_open(psum, m, first):
                    for half in range(2):
                        nc.tensor.matmul(
                            psum[:],
                            we2_t[:, half * D + m * 128:half * D + (m + 1) * 128],
                            hs[half][:], start=(first and half == 0),
                            stop=False)

                def out_tile(m, src_ps, src_ps2=None):
                    yo = op.tile([128, TC], F32, tag='yo', name=f'yo{m}_{c}_{it}')
                    if src_ps2 is None:
                        nc.vector.tensor_copy(yo[:], src_ps[:])
                    else:
                        nc.vector.tensor_tensor(yo[:], src_ps[:], src_ps2[:],
                                                op=ALU.add)
                    # stores ride the ACT HWDGE ring: the sync ring stays
                    # pure-loads so it can prefetch a full iteration ahead
                    nc.scalar.dma_start(
                        yT_h[m * 128:(m + 1) * 128, c0:c0 + TC], yo[:])

                if b2mode == 'rot3':
                    for p in range(2):
                        yps = [ps_y.tile([128, TC], F32, tag='yT',
                                         name=f'yp{p}_{i}_{c}_{it}')
                               for i in range(3)]
                        for m3 in range(3):
                            if hs is not None:
                                moe_open(yps[m3], p * 3 + m3, True)
                        for km in range(KM):
                            for m3 in range(3):
                                nc.tensor.matmul(
                                    yps[m3][:],
                                    w2_t[km][:, (p * 3 + m3) * 128:
                                          (p * 3 + m3 + 1) * 128],
                                    ht[km][:],
                                    start=(hs is None and km == 0),
                                    stop=(km == KM - 1))
                        for m3 in range(3):
                            out_tile(p * 3 + m3, yps[m3])
                elif b2mode == 'serial':
                    for m in range(MD):
                        yp = ps_y.tile([128, TC], F32, tag='yT',
                                       name=f'yp{m}_{c}_{it}')
                        if hs is not None:
                            moe_open(yp, m, True)
                        for km in range(KM):
                            nc.tensor.matmul(
                                yp[:], w2_t[km][:, m * 128:(m + 1) * 128],
                                ht[km][:], start=(hs is None and km == 0),
                                stop=(km == KM - 1))
                        out_tile(m, yp)
                else:  # split: two 13-accum halves per m-tile + DVE add
                    for m in range(MD):
                        ypa = ps_y.tile([128, TC], F32, tag='yT',
                                        name=f'ypa{m}_{c}_{it}')
                        ypb = ps_y.tile([128, TC], F32, tag='yT',
                                        name=f'ypb{m}_{c}_{it}')
                        if hs is not None:
                            moe_open(ypa, m, True)
                        for km in range(KM // 2):
                            nc.tensor.matmul(
                                ypa[:], w2_t[km][:, m * 128:(m + 1) * 128],
                                ht[km][:], start=(hs is None and km == 0),
                                stop=(km == KM // 2 - 1))
                        for km in range(KM // 2, KM):
                            nc.tensor.matmul(
                                ypb[:], w2_t[km][:, m * 128:(m + 1) * 128],
                                ht[km][:], start=(km == KM // 2),
                                stop=(km == KM - 1))
                        out_tile(m, ypa, ypb)

            def emit_body_iso(it):
                """Isolation builds: loads + B1 (+ B2 when phases='b1b2')."""
                def load_x(c):
                    x_t = []
                    for k in range(KD):
                        t = xp.tile([128, TC], BF16, tag=f'x{k}',
                                    name=f'x{k}_{c}_{it}')
                        nc.sync.dma_start(t[:], xT_h[c, k])
                        x_t.append(t)
                    return x_t

                x_cur = load_x(0)
                for c in range(NCH):
                    c0 = c * TC
                    x_t = x_cur
                    x_nxt = load_x(c + 1) if c + 1 < NCH else None
                    ht = []
                    for km in range(KM):
                        hp = ps_h.tile([128, TC], F32, tag='hT',
                                       name=f'hp{km}_{c}_{it}')
                        for k in range(KD):
                            nc.tensor.matmul(
                                hp[:], w1_t[k][:, km * 128:(km + 1) * 128],
                                x_t[k][:], start=(k == 0), stop=(k == KD - 1))
                        hg = htp.tile([128, TC], BF16, tag='ht',
                                      name=f'ht{km}_{c}_{it}')
                        nc.scalar.activation(hg[:], hp[:], AF.Gelu_apprx_tanh)
                        ht.append(hg)
                    if phases == 'b1':
                        # keep ht observable: dump one tile (bitcast bf16->f32)
                        nc.scalar.dma_start(
                            yT_h[0:128, c0:c0 + TC // 2],
                            ht[23][:].bitcast(F32))
                    else:
                        emit_b2(c, ht, None, it)
                    x_cur = x_nxt

            def emit_body(it):
                # ---------- iteration-top loads ----------
                # xg: 24 (c,k) fp32 tiles for the G phase (bufs=1: the DMA
                # queue runs ahead, so these transfer during the previous
                # iteration's chunks).
                xg_t = [[None] * KD for _ in range(NCH)]
                for c in range(NCH):
                    for k in range(KD):
                        t = gp.tile([128, TC], F32R, tag=f'xg{c}_{k}', bufs=1,
                                    name=f'xg{c}_{k}_{it}')
                        nc.sync.dma_start(t[:], xgT_h[c, k])
                        xg_t[c][k] = t
                nz_t = gp.tile([128, NTT * E], F32, tag='nz', bufs=1,
                               name=f'nz_{it}')
                nc.sync.dma_start(
                    nz_t[:].rearrange('p (t e) -> p t e', t=NTT),
                    noise_h.rearrange('(t p) e -> p t e', p=128))

                def load_x(c):
                    x_t = []
                    for k in range(KD):
                        t = xp.tile([128, TC], BF16, tag=f'x{k}',
                                    name=f'x{k}_{c}_{it}')
                        nc.sync.dma_start(t[:], xT_h[c, k])
                        x_t.append(t)
                    return x_t

                x_cur = load_x(0)

                # ---------- G phase: gate logits + top-2 softmax ----------
                lp8 = []
                lsb = gp.tile([8, TPC], F32, tag='lsb', bufs=1, name=f'lsb_{it}')
                for c in range(NCH):
                    lp = ps_a.tile([8, TC], F32, tag='mA', name=f'lp8_{c}_{it}')
                    for k in range(KD):
                        nc.tensor.matmul(
                            lp[:], wg_t[:, k * E:(k + 1) * E], xg_t[c][k][:],
                            start=(k == 0), stop=(k == KD - 1))
                    lp8.append(lp)
                    nc.vector.tensor_copy(lsb[:, c * TC:(c + 1) * TC], lp[:])

                noisy = gp.tile([128, NTT * E], F32, tag='noisy', bufs=1,
                                name=f'noisy_{it}')
                for t in range(NTT):
                    lt = ps_a.tile([128, E], F32, tag='mA', name=f'lt{t}_{it}')
                    nc.tensor.transpose(
                        lt[:], lsb[:, t * 128:(t + 1) * 128], ident[:8, :8])
                    nc.vector.scalar_tensor_tensor(
                        noisy[:, t * E:(t + 1) * E], nz_t[:, t * E:(t + 1) * E],
                        NOISE_STD, lt[:], op0=ALU.mult, op1=ALU.add)

                # top-2 softmax over noisy logits (DVE + one ACT Exp)
                nv = noisy[:].rearrange('p (t e) -> p t e', t=NTT)
                m1 = gp.tile([128, NTT], F32, tag='m1', name=f'm1_{it}')
                nc.vector.tensor_reduce(m1[:], nv, axis=AX.X, op=ALU.max)
                m1b = m1[:].unsqueeze(-1).broadcast_to([128, NTT, E])
                eq = gp.tile([128, NTT * E], F32, tag='eq', bufs=1,
                             name=f'eq_{it}')
                nc.vector.tensor_tensor(
                    eq[:].rearrange('p (t e) -> p t e', t=NTT), nv, m1b,
                    op=ALU.is_equal)
                nm = gp.tile([128, NTT * E], F32, tag='nm', bufs=1,
                             name=f'nm_{it}')
                nc.vector.scalar_tensor_tensor(
                    nm[:].rearrange('p (t e) -> p t e', t=NTT),
                    eq[:].rearrange('p (t e) -> p t e', t=NTT), -1e30, nv,
                    op0=ALU.mult, op1=ALU.add)
                m2 = gp.tile([128, NTT], F32, tag='m2', name=f'm2_{it}')
                nc.vector.tensor_reduce(
                    m2[:], nm[:].rearrange('p (t e) -> p t e', t=NTT),
                    axis=AX.X, op=ALU.max)
                dlt = gp.tile([128, NTT * E], F32, tag='dlt', bufs=1,
                              name=f'dlt_{it}')
                nc.vector.tensor_tensor(
                    dlt[:].rearrange('p (t e) -> p t e', t=NTT), nv, m1b,
                    op=ALU.subtract)
                ex = gp.tile([128, NTT * E], F32, tag='ex', bufs=1,
                             name=f'ex_{it}')
                nc.scalar.activation(ex[:], dlt[:], AF.Exp)
                mask = gp.tile([128, NTT * E], F32, tag='mask', bufs=1,
                               name=f'mask_{it}')
                nc.vector.tensor_tensor(
                    mask[:].rearrange('p (t e) -> p t e', t=NTT), nv,
                    m2[:].unsqueeze(-1).broadcast_to([128, NTT, E]),
                    op=ALU.is_ge)
                u = gp.tile([128, NTT * E], F32, tag='u', bufs=1, name=f'u_{it}')
                nc.vector.tensor_tensor(u[:], ex[:], mask[:], op=ALU.mult)
                s = gp.tile([128, NTT], F32, tag='s', name=f's_{it}')
                nc.vector.tensor_reduce(
                    s[:], u[:].rearrange('p (t e) -> p t e', t=NTT),
                    axis=AX.X, op=ALU.add)
                rs = gp.tile([128, NTT], F32, tag='rs', name=f'rs_{it}')
                nc.vector.reciprocal(rs[:], s[:])
                w = gp.tile([128, NTT * E], F32, tag='w', bufs=1, name=f'w_{it}')
                nc.vector.tensor_tensor(
                    w[:].rearrange('p (t e) -> p t e', t=NTT),
                    u[:].rearrange('p (t e) -> p t e', t=NTT),
                    rs[:].unsqueeze(-1).broadcast_to([128, NTT, E]),
                    op=ALU.mult)
                wT = gp.tile([8, TPC], BF16, tag='wT', bufs=1, name=f'wT_{it}')

                def emit_w_transposes():
                    # emitted after chunk-0 B1: by then the DVE chain is done
                    for t in range(NTT):
                        tp = ps_a.tile([8, 128], F32, tag='mA',
                                       name=f'tp{t}_{it}')
                        nc.tensor.transpose(
                            tp[:], w[:, t * E:(t + 1) * E], ident[:])
                        nc.vector.tensor_copy(
                            wT[:, t * 128:(t + 1) * 128], tp[:])

                # ---------- chunks ----------
                for c in range(NCH):
                    c0 = c * TC
                    x_t = x_cur
                    x_nxt = load_x(c + 1) if c + 1 < NCH else None

                    # B1: hT[km] = gelu(W1eff @ xT)
                    ht = []
                    for km in range(KM):
                        hp = ps_h.tile([128, TC], F32, tag='hT',
                                       name=f'hp{km}_{c}_{it}')
                        for k in range(KD):
                            nc.tensor.matmul(
                                hp[:], w1_t[k][:, km * 128:(km + 1) * 128],
                                x_t[k][:], start=(k == 0), stop=(k == KD - 1))
                        hg = htp.tile([128, TC], BF16, tag='ht',
                                      name=f'ht{km}_{c}_{it}')
                        nc.scalar.activation(hg[:], hp[:], AF.Gelu_apprx_tanh)
                        ht.append(hg)

                    if c == 0:
                        emit_w_transposes()

                    # experts
                    hs = []
                    for half in range(2):
                        hp2 = ps_a.tile([96, TC], F32, tag='mA',
                                        name=f'hc{half}_{c}_{it}')
                        for k in range(KD):
                            nc.tensor.matmul(
                                hp2[:],
                                we1_t[:, k * EI + half * 96:
                                      k * EI + (half + 1) * 96],
                                x_t[k][:], start=(k == 0), stop=(k == KD - 1))
                        hg2 = ep.tile([96, TC], BF16, tag=f'hg{half}',
                                      name=f'hg{half}_{c}_{it}')
                        nc.scalar.activation(hg2[:], hp2[:], AF.Gelu_apprx_tanh)
                        wp = ps_a.tile([96, TC], F32, tag='mA',
                                       name=f'wp{half}_{c}_{it}')
                        nc.tensor.matmul(
                            wp[:], bb_t[:, half * 96:(half + 1) * 96],
                            wT[:, c0:c0 + TC], start=True, stop=True)
                        hsc = ep.tile([96, TC], BF16, tag=f'hs{half}',
                                      name=f'hs{half}_{c}_{it}')
                        nc.vector.tensor_tensor(hsc[:], hg2[:], wp[:],
                                                op=ALU.mult)
                        hs.append(hsc)

                    emit_b2(c, ht, hs, it)

                    x_cur = x_nxt

            body = emit_body if phases == 'full' else emit_body_iso
            if loop_reps is None:
                body(0)
            else:
                with tc.For_i(0, loop_reps, 1,
                              hint_engines=(mybir.EngineType.PE,)) as _:
                    body(0)

    nc.compile()
    return nc


_nc_cache = [None]


def _prep_host(inputs):
    inputs = {k: np.asarray(v) for k, v in inputs.items()}
    x = np.ascontiguousarray(inputs['x'], np.float32).reshape(B * N, D)
    noise = np.ascontiguousarray(inputs['noise'], np.float32).reshape(B * N, E)
    W1eff = (inputs['W1'] + inputs['dW1']).astype(np.float32)   # [MID, D]
    W2eff = (inputs['W2'] + inputs['dW2']).astype(np.float32)   # [D, MID]
    w1T = np.ascontiguousarray(
        W1eff.T.reshape(KD, 128, MID)).astype(NP_BF16)
    w2T = np.ascontiguousarray(
        W2eff.T.reshape(KM, 128, D)).astype(NP_BF16)
    wgT = np.ascontiguousarray(np.asarray(inputs['Wg'], np.float32).T)  # [D, E]
    We1 = np.asarray(inputs['We1'], np.float32)                  # [E, INNER, D]
    We2 = np.asarray(inputs['We2'], np.float32)                  # [E, D, INNER]
    we1T = np.ascontiguousarray(We1.reshape(EI, D).T).astype(NP_BF16)
    we2T = np.ascontiguousarray(
        We2.transpose(0, 2, 1).reshape(EI, D)).astype(NP_BF16)
    bb = np.zeros((E, EI), np.float32)
    for e in range(E):
        bb[e, e * INNER:(e + 1) * INNER] = 1.0
    bb = bb.astype(NP_BF16)
    xT = np.ascontiguousarray(x.T)                               # [D, B*N] f32
    xT_bf = xT.astype(NP_BF16)
    return xT, xT_bf, noise, w1T, w2T, wgT, we1T, we2T, bb


def _make_in_maps(hosts):
    xT, xT_bf, noise, w1T, w2T, wgT, we1T, we2T, bb = hosts
    in_maps = []
    for c in range(NCORES):
        t0 = c * TPC
        xc = np.ascontiguousarray(
            np.ascontiguousarray(xT_bf[:, t0:t0 + TPC])
            .reshape(KD, 128, NCH, TC).transpose(2, 0, 1, 3))
        xgc = np.ascontiguousarray(
            np.ascontiguousarray(xT[:, t0:t0 + TPC])
            .reshape(KD, 128, NCH, TC).transpose(2, 0, 1, 3))
        in_maps.append({
            'xT': xc,
            'xgT': xgc,
            'noise': np.ascontiguousarray(noise[t0:t0 + TPC, :]),
            'w1T': w1T, 'w2T': w2T, 'wgT': wgT,
            'we1T': we1T, 'we2T': we2T, 'bb': bb,
        })
    return in_maps


def kernel(**inputs):
    hosts = _prep_host(inputs)
    if _nc_cache[0] is None:
        _nc_cache[0] = build_core_program()
    nc = _nc_cache[0]

    in_maps = _make_in_maps(hosts)

    from concourse.bass_utils import run_bass_kernel_spmd
    res = run_bass_kernel_spmd(nc, in_maps, core_ids=list(range(NCORES)))
    out = np.empty((B * N, D), np.float32)
    for c in range(NCORES):
        out[c * TPC:(c + 1) * TPC, :] = res.results[c]['yT'].T
    return out.reshape(B, N, D)



# revision 17
# speedup vs baseline: 1.0387x; 1.0387x over previous
"""Trainium2 Bass kernel for nn_FFN_Shared_Plus_TaskLoRA (moe_routing).

Computation (per token x in R^768):
    y   = gelu_tanh(x @ (W1+dW1)^T) @ (W2+dW2)^T          (biases are all zero)
    g   = top2-softmax(x @ Wg^T + 0.1*noise)              (dense [E=8] weights)
    moe = sum_e g_e * gelu_tanh(x @ We1[e]^T) @ We2[e]^T
    out = y + moe
Sharding: data-parallel over tokens, 2048/core, weights replicated.

v3 layout (per core, per 2048-token iteration):
  - G phase (once per iteration): gate logits for all 2048 tokens in
    fp32r (from an fp32 copy of x — bf16 is too coarse for the top-2
    ranking), PE-transpose to token-major, one batched top-2 softmax
    (one Exp per iteration -> only 2 ACT table reloads per iteration),
    combine weights transposed back to expert-major wT [8, 2048] bf16.
    The PE transposes that depend on the DVE softmax chain are emitted
    after chunk-0 B1 so the PE never waits on DVE/ACT.
  - 4 chunks of 512 tokens: B1 hT = gelu(W1 @ xT) (24 bf16 tiles),
    expert hcat/gating, then B2 with per-m-tile serial k-loops
    accumulating moe + W2 @ hT.  W1, W2 both SBUF-resident in bf16.
  - All PSUM->SBUF copies on DVE (ACT runs gelu only; no table thrash).
"""
import os
import sys

sys.path.insert(0, '/opt/trn_rl_repo')
os.environ.setdefault('BASS_NEVER_TRACE', '1')

from contextlib import ExitStack

import numpy as np
import ml_dtypes

import concourse.bacc as bacc
import concourse.tile as tile
from concourse import mybir
from concourse.masks import make_identity

F32 = mybir.dt.float32
F32R = mybir.dt.float32r
BF16 = mybir.dt.bfloat16
NP_BF16 = ml_dtypes.bfloat16
AF = mybir.ActivationFunctionType
ALU = mybir.AluOpType
AX = mybir.AxisListType

B, N, D = 4, 4096, 768
MID = 4 * D              # 3072
E, INNER = 8, 24
EI = E * INNER           # 192
NOISE_STD = 0.1
NCORES = 8
TPC = (B * N) // NCORES  # 2048 tokens per core
TC = 512                 # chunk of tokens
NCH = TPC // TC          # 4 chunks
KD = D // 128            # 6  k-tiles of the D contraction
KM = MID // 128          # 24 k-tiles of the MID contraction
MD = D // 128            # 6  m-tiles of the D output
NTT = TPC // 128         # 16 token sub-tiles per iteration


def build_core_program(loop_reps=None, phases='full', b2mode='rot3',
                       b2exp=None, gate_mode='exp', store_ring='gpsimd',
                       moe_pos='close', ht_pad=0, batch_tp=False):
    """Build the per-core Bass program. If loop_reps is given, the body is
    wrapped in a For_i (timing builds).

    phases: 'full' | 'b1' (loads+B1 only) | 'b1b2' (no gate/experts)
    b2mode: 'rot3' (two 3-m-tile passes, bank rotation) | 'serial'
            (per-m-tile 26-mm runs) | 'split' (two 13-accum halves + DVE add)
            | 'one6' (single pass, 6 banks; b1b2 only)
    b2exp:  timing experiments: 'xsrc' (B2 reads x tiles, wrong output) |
            'nostore' (skip yo DMA) | 'puremm' (skip copy+store too)
    gate_mode: 'sig' (expert-major top2 via max/min tournament + sigmoid
               margin, no PE transposes) | 'exp' (v3 token-major softmax)
    store_ring: 'vector' (trigger follows the copy on the same queue) |
               'scalar' (v3: rides ACT queue; head-of-line blocks gelus)
    moe_pos: 'close' (moe MMs after the W2 k-loop; wT needed late) | 'open'
    """
    nc = bacc.Bacc('TRN2', target_bir_lowering=False, debug=False)

    xT_h = nc.dram_tensor('xT', [NCH, KD, 128, TC], BF16,
                          kind='ExternalInput').ap()
    xgT_h = nc.dram_tensor('xgT', [NCH, KD, 128, TC], F32R,
                           kind='ExternalInput').ap()
    noise_h = nc.dram_tensor('noise', [TPC, E], F32, kind='ExternalInput').ap()
    noiseT_h = nc.dram_tensor('noiseT', [E, TPC], F32,
                              kind='ExternalInput').ap()
    w1_h = nc.dram_tensor('w1T', [KD, 128, MID], BF16,
                          kind='ExternalInput').ap()
    w2_h = nc.dram_tensor('w2T', [KM, 128, D], BF16, kind='ExternalInput').ap()
    wg_h = nc.dram_tensor('wgT', [D, E], F32R, kind='ExternalInput').ap()
    we1_h = nc.dram_tensor('we1T', [D, EI], BF16, kind='ExternalInput').ap()
    we2_h = nc.dram_tensor('we2T', [EI, D], BF16, kind='ExternalInput').ap()
    bb_h = nc.dram_tensor('bb', [E, EI], BF16, kind='ExternalInput').ap()
    yT_h = nc.dram_tensor('yT', [D, TPC], F32, kind='ExternalOutput').ap()

    with tile.TileContext(nc) as tc:
        with ExitStack() as ctx:
            const = ctx.enter_context(tc.tile_pool(name='const', bufs=1))
            xp = ctx.enter_context(tc.tile_pool(name='xp', bufs=2))
            htp = ctx.enter_context(tc.tile_pool(name='htp', bufs=KM))
            gp = ctx.enter_context(tc.tile_pool(name='gp', bufs=2))
            ep = ctx.enter_context(tc.tile_pool(name='ep', bufs=1))
            op = ctx.enter_context(tc.tile_pool(name='op', bufs=3))
            nb_y = {'split': 4, 'one6': 6}.get(b2mode, 3)
            nb_a = {'split': 2, 'one6': 0}.get(b2mode, 3)
            ps_y = ctx.enter_context(tc.tile_pool(name='ps_y', bufs=nb_y, space='PSUM'))
            ps_h = ctx.enter_context(tc.tile_pool(name='ps_h', bufs=2, space='PSUM'))
            if nb_a:
                ps_a = ctx.enter_context(
                    tc.tile_pool(name='ps_a', bufs=nb_a, space='PSUM'))
            else:
                assert phases != 'full', 'one6 needs ps_a for gate/experts'

            ident = const.tile([128, 128], F32, tag='ident')
            make_identity(nc, ident[:])

            wg_t = const.tile([128, KD * E], F32R, tag='wg')
            nc.sync.dma_start(wg_t[:].rearrange('p (k e) -> p k e', k=KD),
                              wg_h.rearrange('(k p) e -> p k e', p=128))

            # ---- resident weights ----
            w1_t = []
            for k in range(KD):
                t = const.tile([128, MID], BF16, tag=f'w1_{k}')
                nc.sync.dma_start(t[:], w1_h[k])
                w1_t.append(t)
            w2_t = []
            for km in range(KM):
                t = const.tile([128, D], BF16, tag=f'w2_{km}')
                nc.sync.dma_start(t[:], w2_h[km])
                w2_t.append(t)
            we1_t = const.tile([128, KD * EI], BF16, tag='we1')
            nc.sync.dma_start(we1_t[:].rearrange('p (k i) -> p k i', k=KD),
                              we1_h.rearrange('(k p) i -> p k i', p=128))
            we2_t = const.tile([96, 2 * D], BF16, tag='we2')
            nc.sync.dma_start(we2_t[:].rearrange('p (k d) -> p k d', k=2),
                              we2_h.rearrange('(k p) d -> p k d', p=96))
            bb_t = const.tile([E, EI], BF16, tag='bb')
            nc.sync.dma_start(bb_t[:], bb_h[:])

            def emit_b2(c, ht, hs, it):
                """yT = [moe +] W2eff @ hT for one chunk; hs=None skips moe."""
                c0 = c * TC
                closing = hs is not None and moe_pos == 'close'
                opening = hs is not None and moe_pos == 'open'

                def moe_open(psum, m, first):
                    for half in range(2):
                        nc.tensor.matmul(
                            psum[:],
                            we2_t[:, half * D + m * 128:half * D + (m + 1) * 128],
                            hs[half][:], start=(first and half == 0),
                            stop=False)

                def moe_close(psum, m):
                    for half in range(2):
                        nc.tensor.matmul(
                            psum[:],
                            we2_t[:, half * D + m * 128:half * D + (m + 1) * 128],
                            hs[half][:], start=False, stop=(half == 1))

                def out_tile(m, src_ps, src_ps2=None):
                    if b2exp == 'puremm':
                        return
                    yo = op.tile([128, TC], F32, tag='yo', name=f'yo{m}_{c}_{it}')
                    if src_ps2 is None:
                        nc.vector.tensor_copy(yo[:], src_ps[:])
                    else:
                        nc.vector.tensor_tensor(yo[:], src_ps[:], src_ps2[:],
                                                op=ALU.add)
                    if b2exp in ('nostore',):
                        return
                    if store_ring == 'gpsimd':
                        # SWDGE on the (idle) Pool engine: no ACT
                        # head-of-line block, sync ring stays pure-loads
                        nc.gpsimd.dma_start(
                            yT_h[m * 128:(m + 1) * 128, c0:c0 + TC], yo[:])
                    else:
                        nc.scalar.dma_start(
                            yT_h[m * 128:(m + 1) * 128, c0:c0 + TC], yo[:])

                if b2mode == 'one6':
                    yps = [ps_y.tile([128, TC], F32, tag='yT',
                                     name=f'yp6_{m}_{c}_{it}')
                           for m in range(MD)]
                    for m in range(MD):
                        if opening:
                            moe_open(yps[m], m, True)
                    for km in range(KM):
                        for m in range(MD):
                            nc.tensor.matmul(
                                yps[m][:],
                                w2_t[km][:, m * 128:(m + 1) * 128],
                                ht[km][:],
                                start=(not opening and km == 0),
                                stop=(not closing and km == KM - 1))
                    for m in range(MD):
                        if closing:
                            moe_close(yps[m], m)
                        out_tile(m, yps[m])
                elif b2mode == 'rot3':
                    for p in range(2):
                        yps = [ps_y.tile([128, TC], F32, tag='yT',
                                         name=f'yp{p}_{i}_{c}_{it}')
                               for i in range(3)]
                        for m3 in range(3):
                            if opening:
                                moe_open(yps[m3], p * 3 + m3, True)
                        for km in range(KM):
                            for m3 in range(3):
                                nc.tensor.matmul(
                                    yps[m3][:],
                                    w2_t[km][:, (p * 3 + m3) * 128:
                                          (p * 3 + m3 + 1) * 128],
                                    ht[km][:],
                                    start=(not opening and km == 0),
                                    stop=(not closing and km == KM - 1))
                        for m3 in range(3):
                            if closing:
                                moe_close(yps[m3], p * 3 + m3)
                            out_tile(p * 3 + m3, yps[m3])
                elif b2mode == 'serial':
                    for m in range(MD):
                        yp = ps_y.tile([128, TC], F32, tag='yT',
                                       name=f'yp{m}_{c}_{it}')
                        if opening:
                            moe_open(yp, m, True)
                        for km in range(KM):
                            nc.tensor.matmul(
                                yp[:], w2_t[km][:, m * 128:(m + 1) * 128],
                                ht[km][:], start=(not opening and km == 0),
                                stop=(not closing and km == KM - 1))
                        if closing:
                            moe_close(yp, m)
                        out_tile(m, yp)
                else:  # split: two 13-accum halves per m-tile + DVE add
                    for m in range(MD):
                        ypa = ps_y.tile([128, TC], F32, tag='yT',
                                        name=f'ypa{m}_{c}_{it}')
                        ypb = ps_y.tile([128, TC], F32, tag='yT',
                                        name=f'ypb{m}_{c}_{it}')
                        if hs is not None:
                            moe_open(ypa, m, True)
                        for km in range(KM // 2):
                            nc.tensor.matmul(
                                ypa[:], w2_t[km][:, m * 128:(m + 1) * 128],
                                ht[km][:], start=(hs is None and km == 0),
                                stop=(km == KM // 2 - 1))
                        for km in range(KM // 2, KM):
                            nc.tensor.matmul(
                                ypb[:], w2_t[km][:, m * 128:(m + 1) * 128],
                                ht[km][:], start=(km == KM // 2),
                                stop=(km == KM - 1))
                        out_tile(m, ypa, ypb)

            def emit_body_iso(it):
                """Isolation builds: loads + B1 (+ B2 when phases='b1b2')."""
                def load_x(c):
                    x_t = []
                    for k in range(KD):
                        t = xp.tile([128, TC], BF16, tag=f'x{k}',
                                    name=f'x{k}_{c}_{it}')
                        nc.sync.dma_start(t[:], xT_h[c, k])
                        x_t.append(t)
                    return x_t

                x_cur = load_x(0)
                for c in range(NCH):
                    c0 = c * TC
                    x_t = x_cur
                    x_nxt = load_x(c + 1) if c + 1 < NCH else None
                    ht = []
                    for km in range(KM):
                        hp = ps_h.tile([128, TC], F32, tag='hT',
                                       name=f'hp{km}_{c}_{it}')
                        for k in range(KD):
                            nc.tensor.matmul(
                                hp[:], w1_t[k][:, km * 128:(km + 1) * 128],
                                x_t[k][:], start=(k == 0), stop=(k == KD - 1))
                        hg = htp.tile([128, TC], BF16, tag='ht',
                                      name=f'ht{km}_{c}_{it}')
                        nc.scalar.activation(hg[:], hp[:], AF.Gelu_apprx_tanh)
                        ht.append(hg)
                    if phases == 'b1':
                        # keep ht observable: dump one tile (bitcast bf16->f32)
                        nc.scalar.dma_start(
                            yT_h[0:128, c0:c0 + TC // 2],
                            ht[23][:].bitcast(F32))
                    elif b2exp == 'xsrc':
                        emit_b2(c, [x_t[km % KD] for km in range(KM)], None, it)
                    else:
                        emit_b2(c, ht, None, it)
                    x_cur = x_nxt

            def emit_body(it):
                # ---------- iteration-top loads ----------
                # xg: 24 (c,k) fp32 tiles for the G phase (bufs=1: the DMA
                # queue runs ahead, so these transfer during the previous
                # iteration's chunks).
                xg_t = [[None] * KD for _ in range(NCH)]
                for c in range(NCH):
                    for k in range(KD):
                        t = gp.tile([128, TC], F32R, tag=f'xg{c}_{k}', bufs=1,
                                    name=f'xg{c}_{k}_{it}')
                        nc.sync.dma_start(t[:], xgT_h[c, k])
                        xg_t[c][k] = t
                nz_t = gp.tile([128, NTT * E], F32, tag='nz', bufs=1,
                               name=f'nz_{it}')
                nc.sync.dma_start(
                    nz_t[:].rearrange('p (t e) -> p t e', t=NTT),
                    noise_h.rearrange('(t p) e -> p t e', p=128))

                def load_x(c):
                    x_t = []
                    for k in range(KD):
                        t = xp.tile([128, TC], BF16, tag=f'x{k}',
                                    name=f'x{k}_{c}_{it}')
                        nc.sync.dma_start(t[:], xT_h[c, k])
                        x_t.append(t)
                    return x_t

                x_cur = load_x(0)

                # ---------- G phase: gate logits + top-2 softmax ----------
                lp8 = []
                lsb = gp.tile([8, TPC], F32, tag='lsb', bufs=1, name=f'lsb_{it}')
                for c in range(NCH):
                    lp = ps_a.tile([8, TC], F32, tag='mA', name=f'lp8_{c}_{it}')
                    for k in range(KD):
                        nc.tensor.matmul(
                            lp[:], wg_t[:, k * E:(k + 1) * E], xg_t[c][k][:],
                            start=(k == 0), stop=(k == KD - 1))
                    lp8.append(lp)
                    nc.vector.tensor_copy(lsb[:, c * TC:(c + 1) * TC], lp[:])

                noisy = gp.tile([128, NTT * E], F32, tag='noisy', bufs=1,
                                name=f'noisy_{it}')
                for t in range(NTT):
                    lt = ps_a.tile([128, E], F32, tag='mA', name=f'lt{t}_{it}')
                    nc.tensor.transpose(
                        lt[:], lsb[:, t * 128:(t + 1) * 128], ident[:8, :8])
                    nc.vector.scalar_tensor_tensor(
                        noisy[:, t * E:(t + 1) * E], nz_t[:, t * E:(t + 1) * E],
                        NOISE_STD, lt[:], op0=ALU.mult, op1=ALU.add)

                # top-2 softmax over noisy logits (DVE + one ACT Exp)
                nv = noisy[:].rearrange('p (t e) -> p t e', t=NTT)
                m1 = gp.tile([128, NTT], F32, tag='m1', name=f'm1_{it}')
                nc.vector.tensor_reduce(m1[:], nv, axis=AX.X, op=ALU.max)
                m1b = m1[:].unsqueeze(-1).broadcast_to([128, NTT, E])
                eq = gp.tile([128, NTT * E], F32, tag='eq', bufs=1,
                             name=f'eq_{it}')
                nc.vector.tensor_tensor(
                    eq[:].rearrange('p (t e) -> p t e', t=NTT), nv, m1b,
                    op=ALU.is_equal)
                nm = gp.tile([128, NTT * E], F32, tag='nm', bufs=1,
                             name=f'nm_{it}')
                nc.vector.scalar_tensor_tensor(
                    nm[:].rearrange('p (t e) -> p t e', t=NTT),
                    eq[:].rearrange('p (t e) -> p t e', t=NTT), -1e30, nv,
                    op0=ALU.mult, op1=ALU.add)
                m2 = gp.tile([128, NTT], F32, tag='m2', name=f'm2_{it}')
                nc.vector.tensor_reduce(
                    m2[:], nm[:].rearrange('p (t e) -> p t e', t=NTT),
                    axis=AX.X, op=ALU.max)
                dlt = gp.tile([128, NTT * E], F32, tag='dlt', bufs=1,
                              name=f'dlt_{it}')
                nc.vector.tensor_tensor(
                    dlt[:].rearrange('p (t e) -> p t e', t=NTT), nv, m1b,
                    op=ALU.subtract)
                ex = gp.tile([128, NTT * E], F32, tag='ex', bufs=1,
                             name=f'ex_{it}')
                nc.scalar.activation(ex[:], dlt[:], AF.Exp)
                mask = gp.tile([128, NTT * E], F32, tag='mask', bufs=1,
                               name=f'mask_{it}')
                nc.vector.tensor_tensor(
                    mask[:].rearrange('p (t e) -> p t e', t=NTT), nv,
                    m2[:].unsqueeze(-1).broadcast_to([128, NTT, E]),
                    op=ALU.is_ge)
                u = gp.tile([128, NTT * E], F32, tag='u', bufs=1, name=f'u_{it}')
                nc.vector.tensor_tensor(u[:], ex[:], mask[:], op=ALU.mult)
                s = gp.tile([128, NTT], F32, tag='s', name=f's_{it}')
                nc.vector.tensor_reduce(
                    s[:], u[:].rearrange('p (t e) -> p t e', t=NTT),
                    axis=AX.X, op=ALU.add)
                rs = gp.tile([128, NTT], F32, tag='rs', name=f'rs_{it}')
                nc.vector.reciprocal(rs[:], s[:])
                w = gp.tile([128, NTT * E], F32, tag='w', bufs=1, name=f'w_{it}')
                nc.vector.tensor_tensor(
                    w[:].rearrange('p (t e) -> p t e', t=NTT),
                    u[:].rearrange('p (t e) -> p t e', t=NTT),
                    rs[:].unsqueeze(-1).broadcast_to([128, NTT, E]),
                    op=ALU.mult)
                wT = gp.tile([8, TPC], BF16, tag='wT', bufs=1, name=f'wT_{it}')

                def emit_w_transposes():
                    # emitted after chunk-0 B1: by then the DVE chain is done
                    for t in range(NTT):
                        tp = ps_a.tile([8, 128], F32, tag='mA',
                                       name=f'tp{t}_{it}')
                        nc.tensor.transpose(
                            tp[:], w[:, t * E:(t + 1) * E], ident[:])
                        nc.vector.tensor_copy(
                            wT[:, t * 128:(t + 1) * 128], tp[:])

                # ---------- chunks ----------
                for c in range(NCH):
                    c0 = c * TC
                    x_t = x_cur
                    x_nxt = load_x(c + 1) if c + 1 < NCH else None

                    # B1: hT[km] = gelu(W1eff @ xT)
                    ht = []
                    for km in range(KM):
                        hp = ps_h.tile([128, TC], F32, tag='hT',
                                       name=f'hp{km}_{c}_{it}')
                        for k in range(KD):
                            nc.tensor.matmul(
                                hp[:], w1_t[k][:, km * 128:(km + 1) * 128],
                                x_t[k][:], start=(k == 0), stop=(k == KD - 1))
                        hg = htp.tile([128, TC], BF16, tag='ht',
                                      name=f'ht{km}_{c}_{it}')
                        nc.scalar.activation(hg[:], hp[:], AF.Gelu_apprx_tanh)
                        ht.append(hg)

                    if c == 0:
                        emit_w_transposes()

                    # experts
                    hs = []
                    for half in range(2):
                        hp2 = ps_a.tile([96, TC], F32, tag='mA',
                                        name=f'hc{half}_{c}_{it}')
                        for k in range(KD):
                            nc.tensor.matmul(
                                hp2[:],
                                we1_t[:, k * EI + half * 96:
                                      k * EI + (half + 1) * 96],
                                x_t[k][:], start=(k == 0), stop=(k == KD - 1))
                        hg2 = ep.tile([96, TC], BF16, tag=f'hg{half}',
                                      name=f'hg{half}_{c}_{it}')
                        nc.scalar.activation(hg2[:], hp2[:], AF.Gelu_apprx_tanh)
                        wp = ps_a.tile([96, TC], F32, tag='mA',
                                       name=f'wp{half}_{c}_{it}')
                        nc.tensor.matmul(
                            wp[:], bb_t[:, half * 96:(half + 1) * 96],
                            wT[:, c0:c0 + TC], start=True, stop=True)
                        hsc = ep.tile([96, TC], BF16, tag=f'hs{half}',
                                      name=f'hs{half}_{c}_{it}')
                        nc.vector.tensor_tensor(hsc[:], hg2[:], wp[:],
                                                op=ALU.mult)
                        hs.append(hsc)

                    emit_b2(c, ht, hs, it)

                    x_cur = x_nxt

            def emit_body_sig(it):
                """v4 G phase: everything expert-major [8, TPC].

                top-2 softmax == sigmoid of the margin:
                  w[e] = 1[v[e] >= m2] * sigmoid(2*v[e] - m1 - m2)
                (for the argmax, 2v-m1-m2 = v1-v2; for the runner-up it is
                v2-v1; all other experts are masked). m1/m2 come from a
                max/min tournament over the 8 partition rows; the only
                partition broadcasts are two exact GPSIMD copies. No PE
                transposes, no token-major layout anywhere.
                """
                xg_t = [[None] * KD for _ in range(NCH)]
                for c in range(NCH):
                    for k in range(KD):
                        t = gp.tile([128, TC], F32R, tag=f'xg{c}_{k}', bufs=1,
                                    name=f'xg{c}_{k}_{it}')
                        nc.sync.dma_start(t[:], xgT_h[c, k])
                        xg_t[c][k] = t

                # row-vector scratch, partition-sliced out of two tiles
                gw = gp.tile([64, TPC], F32, tag='gw', bufs=1, name=f'gw_{it}')
                gbm = gp.tile([16, TPC], BF16, tag='gbm', bufs=1,
                              name=f'gbm_{it}')
                nz = gp.tile([E, TPC], F32, tag='nzT', bufs=1, name=f'nz_{it}')
                wt = gp.tile([E, TPC], BF16, tag='wT', bufs=1, name=f'wt_{it}')
                W = lambda a, b, lo=0, hi=TPC: gw[a:b, lo:hi]
                nc.sync.dma_start(nz[:], noiseT_h[:])

                def load_x(c):
                    x_t = []
                    for k in range(KD):
                        t = xp.tile([128, TC], BF16, tag=f'x{k}',
                                    name=f'x{k}_{c}_{it}')
                        nc.sync.dma_start(t[:], xT_h[c, k])
                        x_t.append(t)
                    return x_t

                x_cur = load_x(0)

                # gate logits + noise, directly expert-major: NOISY = gw[0:8]
                for c in range(NCH):
                    lp = ps_a.tile([8, TC], F32, tag='mA', name=f'lp8_{c}_{it}')
                    for k in range(KD):
                        nc.tensor.matmul(
                            lp[:], wg_t[:, k * E:(k + 1) * E], xg_t[c][k][:],
                            start=(k == 0), stop=(k == KD - 1))
                    nc.vector.scalar_tensor_tensor(
                        W(0, 8, c * TC, (c + 1) * TC),
                        nz[:, c * TC:(c + 1) * TC], NOISE_STD, lp[:],
                        op0=ALU.mult, op1=ALU.add)

                # (max, 2nd-max) tournament across the 8 expert rows
                tt = nc.vector.tensor_tensor
                tt(W(8, 12), W(0, 4), W(4, 8), op=ALU.max)      # R1X
                tt(W(12, 16), W(0, 4), W(4, 8), op=ALU.min)     # R1N
                tt(W(16, 18), W(8, 10), W(10, 12), op=ALU.max)  # R2X
                tt(W(18, 20), W(8, 10), W(10, 12), op=ALU.min)  # R2N
                tt(W(20, 22), W(12, 14), W(14, 16), op=ALU.max)  # R2B
                tt(W(22, 24), W(18, 20), W(20, 22), op=ALU.max)  # R2S
                tt(W(24, 25), W(16, 17), W(17, 18), op=ALU.max)  # M1
                tt(W(25, 26), W(16, 17), W(17, 18), op=ALU.min)  # R3N
                tt(W(26, 27), W(22, 23), W(23, 24), op=ALU.max)  # R3B
                tt(W(27, 28), W(25, 26), W(26, 27), op=ALU.max)  # M2
                tt(W(28, 29), W(24, 25), W(27, 28), op=ALU.add)  # M12
                # exact partition broadcasts on the (idle) GPSIMD engine
                nc.gpsimd.partition_broadcast(W(48, 56), W(28, 29))  # m1+m2
                nc.gpsimd.partition_broadcast(W(56, 64), W(27, 28))  # m2
                # q = 2*noisy - (m1+m2);  mask = noisy >= m2 (bit-exact)
                nc.vector.scalar_tensor_tensor(
                    W(32, 40), W(0, 8), 2.0, W(48, 56),
                    op0=ALU.mult, op1=ALU.subtract)
                tt(gbm[8:16, :], W(0, 8), W(56, 64), op=ALU.is_ge)

                def emit_gate_tail():
                    # after chunk-0 B1 gelus: one Sigmoid table swap per iter
                    nc.scalar.activation(gbm[0:8, :], W(32, 40), AF.Sigmoid)
                    nc.vector.tensor_tensor(wt[:], gbm[0:8, :], gbm[8:16, :],
                                            op=ALU.mult)

                # ---------- chunks ----------
                for c in range(NCH):
                    c0 = c * TC
                    x_t = x_cur
                    x_nxt = load_x(c + 1) if c + 1 < NCH else None

                    ht = []
                    for km in range(KM):
                        hp = ps_h.tile([128, TC], F32, tag='hT',
                                       name=f'hp{km}_{c}_{it}')
                        for k in range(KD):
                            nc.tensor.matmul(
                                hp[:], w1_t[k][:, km * 128:(km + 1) * 128],
                                x_t[k][:], start=(k == 0), stop=(k == KD - 1))
                        hg = htp.tile([128, TC], BF16, tag='ht',
                                      name=f'ht{km}_{c}_{it}')
                        nc.scalar.activation(hg[:], hp[:], AF.Gelu_apprx_tanh)
                        ht.append(hg)

                    if c == 0:
                        emit_gate_tail()

                    # experts
                    hs = []
                    for half in range(2):
                        hp2 = ps_a.tile([96, TC], F32, tag='mA',
                                        name=f'hc{half}_{c}_{it}')
                        for k in range(KD):
                            nc.tensor.matmul(
                                hp2[:],
                                we1_t[:, k * EI + half * 96:
                                      k * EI + (half + 1) * 96],
                                x_t[k][:], start=(k == 0), stop=(k == KD - 1))
                        hg2 = ep.tile([96, TC], BF16, tag=f'hg{half}',
                                      name=f'hg{half}_{c}_{it}')
                        nc.scalar.activation(hg2[:], hp2[:], AF.Gelu_apprx_tanh)
                        wp = ps_a.tile([96, TC], F32, tag='mA',
                                       name=f'wp{half}_{c}_{it}')
                        nc.tensor.matmul(
                            wp[:], bb_t[:, half * 96:(half + 1) * 96],
                            wt[:, c0:c0 + TC], start=True, stop=True)
                        hsc = ep.tile([96, TC], BF16, tag=f'hs{half}',
                                      name=f'hs{half}_{c}_{it}')
                        nc.vector.tensor_tensor(hsc[:], hg2[:], wp[:],
                                                op=ALU.mult)
                        hs.append(hsc)

                    emit_b2(c, ht, hs, it)

                    x_cur = x_nxt

            if phases != 'full':
                body = emit_body_iso
            elif gate_mode == 'sig':
                body = emit_body_sig
            else:
                body = emit_body
            if loop_reps is None:
                body(0)
            else:
                with tc.For_i(0, loop_reps, 1,
                              hint_engines=(mybir.EngineType.PE,)) as _:
                    body(0)

    nc.compile()
    return nc


_nc_cache = [None]


def _prep_host(inputs):
    inputs = {k: np.asarray(v) for k, v in inputs.items()}
    x = np.ascontiguousarray(inputs['x'], np.float32).reshape(B * N, D)
    noise = np.ascontiguousarray(inputs['noise'], np.float32).reshape(B * N, E)
    W1eff = (inputs['W1'] + inputs['dW1']).astype(np.float32)   # [MID, D]
    W2eff = (inputs['W2'] + inputs['dW2']).astype(np.float32)   # [D, MID]
    w1T = np.ascontiguousarray(
        W1eff.T.reshape(KD, 128, MID)).astype(NP_BF16)
    w2T = np.ascontiguousarray(
        W2eff.T.reshape(KM, 128, D)).astype(NP_BF16)
    wgT = np.ascontiguousarray(np.asarray(inputs['Wg'], np.float32).T)  # [D, E]
    We1 = np.asarray(inputs['We1'], np.float32)                  # [E, INNER, D]
    We2 = np.asarray(inputs['We2'], np.float32)                  # [E, D, INNER]
    we1T = np.ascontiguousarray(We1.reshape(EI, D).T).astype(NP_BF16)
    we2T = np.ascontiguousarray(
        We2.transpose(0, 2, 1).reshape(EI, D)).astype(NP_BF16)
    bb = np.zeros((E, EI), np.float32)
    for e in range(E):
        bb[e, e * INNER:(e + 1) * INNER] = 1.0
    bb = bb.astype(NP_BF16)
    xT = np.ascontiguousarray(x.T)                               # [D, B*N] f32
    xT_bf = xT.astype(NP_BF16)
    return xT, xT_bf, noise, w1T, w2T, wgT, we1T, we2T, bb


def _make_in_maps(hosts):
    xT, xT_bf, noise, w1T, w2T, wgT, we1T, we2T, bb = hosts
    in_maps = []
    for c in range(NCORES):
        t0 = c * TPC
        xc = np.ascontiguousarray(
            np.ascontiguousarray(xT_bf[:, t0:t0 + TPC])
            .reshape(KD, 128, NCH, TC).transpose(2, 0, 1, 3))
        xgc = np.ascontiguousarray(
            np.ascontiguousarray(xT[:, t0:t0 + TPC])
            .reshape(KD, 128, NCH, TC).transpose(2, 0, 1, 3))
        in_maps.append({
            'xT': xc,
            'xgT': xgc,
            'noise': np.ascontiguousarray(noise[t0:t0 + TPC, :]),
            'noiseT': np.ascontiguousarray(noise[t0:t0 + TPC, :].T),
            'w1T': w1T, 'w2T': w2T, 'wgT': wgT,
            'we1T': we1T, 'we2T': we2T, 'bb': bb,
        })
    return in_maps


def kernel(**inputs):
    hosts = _prep_host(inputs)
    if _nc_cache[0] is None:
        _nc_cache[0] = build_core_program()
    nc = _nc_cache[0]

    in_maps = _make_in_maps(hosts)

    from concourse.bass_utils import run_bass_kernel_spmd
    res = run_bass_kernel_spmd(nc, in_maps, core_ids=list(range(NCORES)))
    out = np.empty((B * N, D), np.float32)
    for c in range(NCORES):
        out[c * TPC:(c + 1) * TPC, :] = res.results[c]['yT'].T
    return out.reshape(B, N, D)



# revision 23
# speedup vs baseline: 1.0443x; 1.0053x over previous
"""Trainium2 Bass kernel for nn_FFN_Shared_Plus_TaskLoRA (moe_routing).

Computation (per token x in R^768):
    y   = gelu_tanh(x @ (W1+dW1)^T) @ (W2+dW2)^T          (biases are all zero)
    g   = top2-softmax(x @ Wg^T + 0.1*noise)              (dense [E=8] weights)
    moe = sum_e g_e * gelu_tanh(x @ We1[e]^T) @ We2[e]^T
    out = y + moe
Sharding: data-parallel over tokens, 2048/core, weights replicated.

v4 layout (per core, per 2048-token iteration):
  - G phase (once per iteration): gate logits for all 2048 tokens in
    fp32r (from an fp32 copy of x — bf16 is too coarse for the top-2
    ranking), PE-transpose to token-major, one batched top-2 softmax
    (one Exp per iteration -> only 2 ACT table reloads per iteration),
    combine weights transposed back to expert-major wT [8, 2048] bf16.
    Transposes are PACKED 8 (fwd) / 4 (back) per PSUM tile so the PE
    emits them back-to-back with 2+4 DVE ops total (v3 ping-ponged
    PE<->DVE per 128-token tile, ~8us of PE gaps on HW).
  - 4 chunks of 512 tokens: B1 hT = gelu(W1 @ xT) (24 bf16 tiles,
    free-dim padded to skew SBUF banks vs the w2 LDW stream), expert
    hcat/gating, then B2 rot3 (two 3-m-tile PSUM-rotating passes) with
    the moe We2-MMs CLOSING each accumulation group (start on the W2
    k-loop) so B2 never waits on the expert hcat chain.
  - PSUM->SBUF copies on DVE; yT stores ride the GPSIMD SWDGE ring
    (v3 put them on the ACT HWDGE ring, where the triggers head-of-line
    blocked the next chunk's gelus behind mid-B2 DVE copies).
  - measured per-iteration (2048 tok/core, 8 cores): v3 365us -> v4
    343us; PE-busy floor for this shape is ~279us (bf16, 512-wide MMs).
"""
import os
import sys

sys.path.insert(0, '/opt/trn_rl_repo')
os.environ.setdefault('BASS_NEVER_TRACE', '1')

from contextlib import ExitStack

import numpy as np
import ml_dtypes

import concourse.bacc as bacc
import concourse.tile as tile
from concourse import mybir
from concourse.masks import make_identity

F32 = mybir.dt.float32
F32R = mybir.dt.float32r
BF16 = mybir.dt.bfloat16
NP_BF16 = ml_dtypes.bfloat16
AF = mybir.ActivationFunctionType
ALU = mybir.AluOpType
AX = mybir.AxisListType

B, N, D = 4, 4096, 768
MID = 4 * D              # 3072
E, INNER = 8, 24
EI = E * INNER           # 192
NOISE_STD = 0.1
NCORES = 8
TPC = (B * N) // NCORES  # 2048 tokens per core
TC = 512                 # chunk of tokens
NCH = TPC // TC          # 4 chunks
KD = D // 128            # 6  k-tiles of the D contraction
KM = MID // 128          # 24 k-tiles of the MID contraction
MD = D // 128            # 6  m-tiles of the D output
NTT = TPC // 128         # 16 token sub-tiles per iteration


def build_core_program(loop_reps=None, phases='full', b2mode='rot3',
                       b2exp=None, gate_mode='exp', store_ring='gpsimd',
                       moe_pos='close', ht_pad=64, batch_tp=True):
    """Build the per-core Bass program. If loop_reps is given, the body is
    wrapped in a For_i (timing builds).

    phases: 'full' | 'b1' (loads+B1 only) | 'b1b2' (no gate/experts)
    b2mode: 'rot3' (two 3-m-tile passes, bank rotation) | 'serial'
            (per-m-tile 26-mm runs) | 'split' (two 13-accum halves + DVE add)
            | 'one6' (single pass, 6 banks; b1b2 only)
    b2exp:  timing experiments: 'xsrc' (B2 reads x tiles, wrong output) |
            'nostore' (skip yo DMA) | 'puremm' (skip copy+store too)
    gate_mode: 'sig' (expert-major top2 via max/min tournament + sigmoid
               margin, no PE transposes) | 'exp' (v3 token-major softmax)
    store_ring: 'vector' (trigger follows the copy on the same queue) |
               'scalar' (v3: rides ACT queue; head-of-line blocks gelus)
    moe_pos: 'close' (moe MMs after the W2 k-loop; wT needed late) | 'open'
    """
    nc = bacc.Bacc('TRN2', target_bir_lowering=False, debug=False)

    xT_h = nc.dram_tensor('xT', [NCH, KD, 128, TC], BF16,
                          kind='ExternalInput').ap()
    xgT_h = nc.dram_tensor('xgT', [NCH, KD, 128, TC], F32R,
                           kind='ExternalInput').ap()
    noise_h = nc.dram_tensor('noise', [TPC, E], F32, kind='ExternalInput').ap()
    noiseT_h = nc.dram_tensor('noiseT', [E, TPC], F32,
                              kind='ExternalInput').ap()
    w1_h = nc.dram_tensor('w1T', [KD, 128, MID], BF16,
                          kind='ExternalInput').ap()
    w2_h = nc.dram_tensor('w2T', [KM, 128, D], BF16, kind='ExternalInput').ap()
    wg_h = nc.dram_tensor('wgT', [D, E], F32R, kind='ExternalInput').ap()
    we1_h = nc.dram_tensor('we1T', [D, EI], BF16, kind='ExternalInput').ap()
    we2_h = nc.dram_tensor('we2T', [EI, D], BF16, kind='ExternalInput').ap()
    bb_h = nc.dram_tensor('bb', [E, EI], BF16, kind='ExternalInput').ap()
    yT_h = nc.dram_tensor('yT', [D, TPC], F32, kind='ExternalOutput').ap()

    with tile.TileContext(nc) as tc:
        with ExitStack() as ctx:
            const = ctx.enter_context(tc.tile_pool(name='const', bufs=1))
            xp = ctx.enter_context(tc.tile_pool(name='xp', bufs=2))
            htp = ctx.enter_context(tc.tile_pool(name='htp', bufs=KM))
            gp = ctx.enter_context(tc.tile_pool(name='gp', bufs=2))
            ep = ctx.enter_context(tc.tile_pool(name='ep', bufs=1))
            op = ctx.enter_context(tc.tile_pool(name='op', bufs=3))
            nb_y = {'split': 4, 'one6': 6}.get(b2mode, 3)
            nb_a = {'split': 2, 'one6': 0}.get(b2mode, 3)
            ps_y = ctx.enter_context(tc.tile_pool(name='ps_y', bufs=nb_y, space='PSUM'))
            ps_h = ctx.enter_context(tc.tile_pool(name='ps_h', bufs=2, space='PSUM'))
            if nb_a:
                ps_a = ctx.enter_context(
                    tc.tile_pool(name='ps_a', bufs=nb_a, space='PSUM'))
            else:
                assert phases != 'full', 'one6 needs ps_a for gate/experts'

            ident = const.tile([128, 128], F32, tag='ident')
            make_identity(nc, ident[:])

            wg_t = const.tile([128, KD * E], F32R, tag='wg')
            nc.sync.dma_start(wg_t[:].rearrange('p (k e) -> p k e', k=KD),
                              wg_h.rearrange('(k p) e -> p k e', p=128))

            # ---- resident weights ----
            w1_t = []
            for k in range(KD):
                t = const.tile([128, MID], BF16, tag=f'w1_{k}')
                nc.sync.dma_start(t[:], w1_h[k])
                w1_t.append(t)
            w2_t = []
            for km in range(KM):
                t = const.tile([128, D], BF16, tag=f'w2_{km}')
                nc.sync.dma_start(t[:], w2_h[km])
                w2_t.append(t)
            we1_t = const.tile([128, KD * EI], BF16, tag='we1')
            nc.sync.dma_start(we1_t[:].rearrange('p (k i) -> p k i', k=KD),
                              we1_h.rearrange('(k p) i -> p k i', p=128))
            we2_t = const.tile([96, 2 * D], BF16, tag='we2')
            nc.sync.dma_start(we2_t[:].rearrange('p (k d) -> p k d', k=2),
                              we2_h.rearrange('(k p) d -> p k d', p=96))
            bb_t = const.tile([E, EI], BF16, tag='bb')
            nc.sync.dma_start(bb_t[:], bb_h[:])

            def emit_b2(c, ht, hs, it):
                """yT = [moe +] W2eff @ hT for one chunk; hs=None skips moe."""
                c0 = c * TC
                closing = hs is not None and moe_pos == 'close'
                opening = hs is not None and moe_pos == 'open'

                def moe_open(psum, m, first):
                    for half in range(2):
                        nc.tensor.matmul(
                            psum[:],
                            we2_t[:, half * D + m * 128:half * D + (m + 1) * 128],
                            hs[half][:], start=(first and half == 0),
                            stop=False)

                def moe_close(psum, m):
                    for half in range(2):
                        nc.tensor.matmul(
                            psum[:],
                            we2_t[:, half * D + m * 128:half * D + (m + 1) * 128],
                            hs[half][:], start=False, stop=(half == 1))

                def out_tile(m, src_ps, src_ps2=None):
                    if b2exp == 'puremm':
                        return
                    yo = op.tile([128, TC], F32, tag='yo', name=f'yo{m}_{c}_{it}')
                    if src_ps2 is None:
                        nc.vector.tensor_copy(yo[:], src_ps[:])
                    else:
                        nc.vector.tensor_tensor(yo[:], src_ps[:], src_ps2[:],
                                                op=ALU.add)
                    if b2exp in ('nostore',):
                        return
                    if store_ring == 'gpsimd':
                        # SWDGE on the (idle) Pool engine: no ACT
                        # head-of-line block, sync ring stays pure-loads
                        nc.gpsimd.dma_start(
                            yT_h[m * 128:(m + 1) * 128, c0:c0 + TC], yo[:])
                    else:
                        nc.scalar.dma_start(
                            yT_h[m * 128:(m + 1) * 128, c0:c0 + TC], yo[:])

                if b2mode == 'one6':
                    yps = [ps_y.tile([128, TC], F32, tag='yT',
                                     name=f'yp6_{m}_{c}_{it}')
                           for m in range(MD)]
                    for m in range(MD):
                        if opening:
                            moe_open(yps[m], m, True)
                    for km in range(KM):
                        for m in range(MD):
                            nc.tensor.matmul(
                                yps[m][:],
                                w2_t[km][:, m * 128:(m + 1) * 128],
                                ht[km][:, :TC],
                                start=(not opening and km == 0),
                                stop=(not closing and km == KM - 1))
                    for m in range(MD):
                        if closing:
                            moe_close(yps[m], m)
                        out_tile(m, yps[m])
                elif b2mode == 'rot3':
                    for p in range(2):
                        yps = [ps_y.tile([128, TC], F32, tag='yT',
                                         name=f'yp{p}_{i}_{c}_{it}')
                               for i in range(3)]
                        for m3 in range(3):
                            if opening:
                                moe_open(yps[m3], p * 3 + m3, True)
                        for km in range(KM):
                            for m3 in range(3):
                                nc.tensor.matmul(
                                    yps[m3][:],
                                    w2_t[km][:, (p * 3 + m3) * 128:
                                          (p * 3 + m3 + 1) * 128],
                                    ht[km][:, :TC],
                                    start=(not opening and km == 0),
                                    stop=(not closing and km == KM - 1))
                        for m3 in range(3):
                            if closing:
                                moe_close(yps[m3], p * 3 + m3)
                            out_tile(p * 3 + m3, yps[m3])
                elif b2mode == 'serial':
                    for m in range(MD):
                        yp = ps_y.tile([128, TC], F32, tag='yT',
                                       name=f'yp{m}_{c}_{it}')
                        if opening:
                            moe_open(yp, m, True)
                        for km in range(KM):
                            nc.tensor.matmul(
                                yp[:], w2_t[km][:, m * 128:(m + 1) * 128],
                                ht[km][:, :TC], start=(not opening and km == 0),
                                stop=(not closing and km == KM - 1))
                        if closing:
                            moe_close(yp, m)
                        out_tile(m, yp)
                else:  # split: two 13-accum halves per m-tile + DVE add
                    for m in range(MD):
                        ypa = ps_y.tile([128, TC], F32, tag='yT',
                                        name=f'ypa{m}_{c}_{it}')
                        ypb = ps_y.tile([128, TC], F32, tag='yT',
                                        name=f'ypb{m}_{c}_{it}')
                        if hs is not None:
                            moe_open(ypa, m, True)
                        for km in range(KM // 2):
                            nc.tensor.matmul(
                                ypa[:], w2_t[km][:, m * 128:(m + 1) * 128],
                                ht[km][:, :TC], start=(hs is None and km == 0),
                                stop=(km == KM // 2 - 1))
                        for km in range(KM // 2, KM):
                            nc.tensor.matmul(
                                ypb[:], w2_t[km][:, m * 128:(m + 1) * 128],
                                ht[km][:, :TC], start=(km == KM // 2),
                                stop=(km == KM - 1))
                        out_tile(m, ypa, ypb)

            def emit_body_iso(it):
                """Isolation builds: loads + B1 (+ B2 when phases='b1b2')."""
                def load_x(c):
                    x_t = []
                    for k in range(KD):
                        t = xp.tile([128, TC], BF16, tag=f'x{k}',
                                    name=f'x{k}_{c}_{it}')
                        nc.sync.dma_start(t[:], xT_h[c, k])
                        x_t.append(t)
                    return x_t

                x_cur = load_x(0)
                for c in range(NCH):
                    c0 = c * TC
                    x_t = x_cur
                    x_nxt = load_x(c + 1) if c + 1 < NCH else None
                    ht = []
                    for km in range(KM):
                        hp = ps_h.tile([128, TC], F32, tag='hT',
                                       name=f'hp{km}_{c}_{it}')
                        for k in range(KD):
                            nc.tensor.matmul(
                                hp[:], w1_t[k][:, km * 128:(km + 1) * 128],
                                x_t[k][:], start=(k == 0), stop=(k == KD - 1))
                        hg = htp.tile([128, TC + ht_pad], BF16, tag='ht',
                                      name=f'ht{km}_{c}_{it}')
                        nc.scalar.activation(hg[:, :TC], hp[:],
                                             AF.Gelu_apprx_tanh)
                        ht.append(hg)
                    if phases == 'b1':
                        # keep ht observable: dump one tile (bitcast bf16->f32)
                        nc.scalar.dma_start(
                            yT_h[0:128, c0:c0 + TC // 2],
                            ht[23][:, :TC].bitcast(F32))
                    elif b2exp == 'xsrc':
                        emit_b2(c, [x_t[km % KD] for km in range(KM)], None, it)
                    else:
                        emit_b2(c, ht, None, it)
                    x_cur = x_nxt

            def emit_body(it):
                # ---------- iteration-top loads ----------
                # xg: 24 (c,k) fp32 tiles for the G phase (bufs=1: the DMA
                # queue runs ahead, so these transfer during the previous
                # iteration's chunks).
                xg_t = [[None] * KD for _ in range(NCH)]
                for c in range(NCH):
                    for k in range(KD):
                        t = gp.tile([128, TC], F32R, tag=f'xg{c}_{k}', bufs=1,
                                    name=f'xg{c}_{k}_{it}')
                        nc.sync.dma_start(t[:], xgT_h[c, k])
                        xg_t[c][k] = t
                nz_t = gp.tile([128, NTT * E], F32, tag='nz', bufs=1,
                               name=f'nz_{it}')
                nc.sync.dma_start(
                    nz_t[:].rearrange('p (t e) -> p t e', t=NTT),
                    noise_h.rearrange('(t p) e -> p t e', p=128))

                def load_x(c):
                    x_t = []
                    for k in range(KD):
                        t = xp.tile([128, TC], BF16, tag=f'x{k}',
                                    name=f'x{k}_{c}_{it}')
                        nc.sync.dma_start(t[:], xT_h[c, k])
                        x_t.append(t)
                    return x_t

                x_cur = load_x(0)

                # ---------- G phase: gate logits + top-2 softmax ----------
                lp8 = []
                lsb = gp.tile([8, TPC], F32, tag='lsb', bufs=1, name=f'lsb_{it}')
                for c in range(NCH):
                    lp = ps_a.tile([8, TC], F32, tag='mA', name=f'lp8_{c}_{it}')
                    for k in range(KD):
                        nc.tensor.matmul(
                            lp[:], wg_t[:, k * E:(k + 1) * E], xg_t[c][k][:],
                            start=(k == 0), stop=(k == KD - 1))
                    lp8.append(lp)
                    nc.vector.tensor_copy(lsb[:, c * TC:(c + 1) * TC], lp[:])

                noisy = gp.tile([128, NTT * E], F32, tag='noisy', bufs=1,
                                name=f'noisy_{it}')
                if batch_tp:
                    # pack 8 transposes per PSUM tile: 2 DVE stt ops total
                    for g in range(2):
                        lt8 = ps_a.tile([128, 8 * E], F32, tag='mA',
                                        name=f'lt8_{g}_{it}')
                        for t8 in range(8):
                            t = g * 8 + t8
                            nc.tensor.transpose(
                                lt8[:, t8 * E:(t8 + 1) * E],
                                lsb[:, t * 128:(t + 1) * 128], ident[:8, :8])
                        nc.vector.scalar_tensor_tensor(
                            noisy[:, g * 8 * E:(g + 1) * 8 * E],
                            nz_t[:, g * 8 * E:(g + 1) * 8 * E],
                            NOISE_STD, lt8[:], op0=ALU.mult, op1=ALU.add)
                else:
                    for t in range(NTT):
                        lt = ps_a.tile([128, E], F32, tag='mA',
                                       name=f'lt{t}_{it}')
                        nc.tensor.transpose(
                            lt[:], lsb[:, t * 128:(t + 1) * 128], ident[:8, :8])
                        nc.vector.scalar_tensor_tensor(
                            noisy[:, t * E:(t + 1) * E],
                            nz_t[:, t * E:(t + 1) * E],
                            NOISE_STD, lt[:], op0=ALU.mult, op1=ALU.add)

                # top-2 softmax over noisy logits (DVE + one ACT Exp)
                nv = noisy[:].rearrange('p (t e) -> p t e', t=NTT)
                m1 = gp.tile([128, NTT], F32, tag='m1', name=f'm1_{it}')
                nc.vector.tensor_reduce(m1[:], nv, axis=AX.X, op=ALU.max)
                m1b = m1[:].unsqueeze(-1).broadcast_to([128, NTT, E])
                eq = gp.tile([128, NTT * E], F32, tag='eq', bufs=1,
                             name=f'eq_{it}')
                nc.vector.tensor_tensor(
                    eq[:].rearrange('p (t e) -> p t e', t=NTT), nv, m1b,
                    op=ALU.is_equal)
                nm = gp.tile([128, NTT * E], F32, tag='nm', bufs=1,
                             name=f'nm_{it}')
                nc.vector.scalar_tensor_tensor(
                    nm[:].rearrange('p (t e) -> p t e', t=NTT),
                    eq[:].rearrange('p (t e) -> p t e', t=NTT), -1e30, nv,
                    op0=ALU.mult, op1=ALU.add)
                m2 = gp.tile([128, NTT], F32, tag='m2', name=f'm2_{it}')
                nc.vector.tensor_reduce(
                    m2[:], nm[:].rearrange('p (t e) -> p t e', t=NTT),
                    axis=AX.X, op=ALU.max)
                dlt = gp.tile([128, NTT * E], F32, tag='dlt', bufs=1,
                              name=f'dlt_{it}')
                nc.vector.tensor_tensor(
                    dlt[:].rearrange('p (t e) -> p t e', t=NTT), nv, m1b,
                    op=ALU.subtract)
                ex = gp.tile([128, NTT * E], F32, tag='ex', bufs=1,
                             name=f'ex_{it}')
                nc.scalar.activation(ex[:], dlt[:], AF.Exp)
                mask = gp.tile([128, NTT * E], F32, tag='mask', bufs=1,
                               name=f'mask_{it}')
                nc.vector.tensor_tensor(
                    mask[:].rearrange('p (t e) -> p t e', t=NTT), nv,
                    m2[:].unsqueeze(-1).broadcast_to([128, NTT, E]),
                    op=ALU.is_ge)
                u = gp.tile([128, NTT * E], F32, tag='u', bufs=1, name=f'u_{it}')
                nc.vector.tensor_tensor(u[:], ex[:], mask[:], op=ALU.mult)
                s = gp.tile([128, NTT], F32, tag='s', name=f's_{it}')
                nc.vector.tensor_reduce(
                    s[:], u[:].rearrange('p (t e) -> p t e', t=NTT),
                    axis=AX.X, op=ALU.add)
                rs = gp.tile([128, NTT], F32, tag='rs', name=f'rs_{it}')
                nc.vector.reciprocal(rs[:], s[:])
                w = gp.tile([128, NTT * E], F32, tag='w', bufs=1, name=f'w_{it}')
                nc.vector.tensor_tensor(
                    w[:].rearrange('p (t e) -> p t e', t=NTT),
                    u[:].rearrange('p (t e) -> p t e', t=NTT),
                    rs[:].unsqueeze(-1).broadcast_to([128, NTT, E]),
                    op=ALU.mult)
                wT = gp.tile([8, TPC], BF16, tag='wT', bufs=1, name=f'wT_{it}')

                def emit_w_transposes():
                    # emitted after chunk-0 B1: by then the DVE chain is done
                    if batch_tp:
                        # pack 4 transposes per PSUM tile: 4 DVE copies total
                        for g in range(4):
                            tp4 = ps_a.tile([8, 4 * 128], F32, tag='mA',
                                            name=f'tp4_{g}_{it}')
                            for t4 in range(4):
                                t = g * 4 + t4
                                nc.tensor.transpose(
                                    tp4[:, t4 * 128:(t4 + 1) * 128],
                                    w[:, t * E:(t + 1) * E], ident[:])
                            nc.vector.tensor_copy(
                                wT[:, g * 512:(g + 1) * 512], tp4[:])
                    else:
                        for t in range(NTT):
                            tp = ps_a.tile([8, 128], F32, tag='mA',
                                           name=f'tp{t}_{it}')
                            nc.tensor.transpose(
                                tp[:], w[:, t * E:(t + 1) * E], ident[:])
                            nc.vector.tensor_copy(
                                wT[:, t * 128:(t + 1) * 128], tp[:])

                # ---------- chunks ----------
                for c in range(NCH):
                    c0 = c * TC
                    x_t = x_cur
                    x_nxt = load_x(c + 1) if c + 1 < NCH else None

                    # B1: hT[km] = gelu(W1eff @ xT)
                    ht = []
                    for km in range(KM):
                        hp = ps_h.tile([128, TC], F32, tag='hT',
                                       name=f'hp{km}_{c}_{it}')
                        for k in range(KD):
                            nc.tensor.matmul(
                                hp[:], w1_t[k][:, km * 128:(km + 1) * 128],
                                x_t[k][:], start=(k == 0), stop=(k == KD - 1))
                        hg = htp.tile([128, TC + ht_pad], BF16, tag='ht',
                                      name=f'ht{km}_{c}_{it}')
                        nc.scalar.activation(hg[:, :TC], hp[:],
                                             AF.Gelu_apprx_tanh)
                        ht.append(hg)

                    if c == 0:
                        emit_w_transposes()

                    # experts
                    hs = []
                    for half in range(2):
                        hp2 = ps_a.tile([96, TC], F32, tag='mA',
                                        name=f'hc{half}_{c}_{it}')
                        for k in range(KD):
                            nc.tensor.matmul(
                                hp2[:],
                                we1_t[:, k * EI + half * 96:
                                      k * EI + (half + 1) * 96],
                                x_t[k][:], start=(k == 0), stop=(k == KD - 1))
                        hg2 = ep.tile([96, TC], BF16, tag=f'hg{half}',
                                      name=f'hg{half}_{c}_{it}')
                        nc.scalar.activation(hg2[:], hp2[:], AF.Gelu_apprx_tanh)
                        wp = ps_a.tile([96, TC], F32, tag='mA',
                                       name=f'wp{half}_{c}_{it}')
                        nc.tensor.matmul(
                            wp[:], bb_t[:, half * 96:(half + 1) * 96],
                            wT[:, c0:c0 + TC], start=True, stop=True)
                        hsc = ep.tile([96, TC], BF16, tag=f'hs{half}',
                                      name=f'hs{half}_{c}_{it}')
                        nc.vector.tensor_tensor(hsc[:], hg2[:], wp[:],
                                                op=ALU.mult)
                        hs.append(hsc)

                    emit_b2(c, ht, hs, it)

                    x_cur = x_nxt

            def emit_body_sig(it):
                """v4 G phase: everything expert-major [8, TPC].

                top-2 softmax == sigmoid of the margin:
                  w[e] = 1[v[e] >= m2] * sigmoid(2*v[e] - m1 - m2)
                (for the argmax, 2v-m1-m2 = v1-v2; for the runner-up it is
                v2-v1; all other experts are masked). m1/m2 come from a
                max/min tournament over the 8 partition rows; the only
                partition broadcasts are two exact GPSIMD copies. No PE
                transposes, no token-major layout anywhere.
                """
                xg_t = [[None] * KD for _ in range(NCH)]
                for c in range(NCH):
                    for k in range(KD):
                        t = gp.tile([128, TC], F32R, tag=f'xg{c}_{k}', bufs=1,
                                    name=f'xg{c}_{k}_{it}')
                        nc.sync.dma_start(t[:], xgT_h[c, k])
                        xg_t[c][k] = t

                # row-vector scratch, partition-sliced out of two tiles
                gw = gp.tile([64, TPC], F32, tag='gw', bufs=1, name=f'gw_{it}')
                gbm = gp.tile([16, TPC], BF16, tag='gbm', bufs=1,
                              name=f'gbm_{it}')
                nz = gp.tile([E, TPC], F32, tag='nzT', bufs=1, name=f'nz_{it}')
                wt = gp.tile([E, TPC], BF16, tag='wT', bufs=1, name=f'wt_{it}')
                W = lambda a, b, lo=0, hi=TPC: gw[a:b, lo:hi]
                nc.sync.dma_start(nz[:], noiseT_h[:])

                def load_x(c):
                    x_t = []
                    for k in range(KD):
                        t = xp.tile([128, TC], BF16, tag=f'x{k}',
                                    name=f'x{k}_{c}_{it}')
                        nc.sync.dma_start(t[:], xT_h[c, k])
                        x_t.append(t)
                    return x_t

                x_cur = load_x(0)

                # gate logits + noise, directly expert-major: NOISY = gw[0:8]
                for c in range(NCH):
                    lp = ps_a.tile([8, TC], F32, tag='mA', name=f'lp8_{c}_{it}')
                    for k in range(KD):
                        nc.tensor.matmul(
                            lp[:], wg_t[:, k * E:(k + 1) * E], xg_t[c][k][:],
                            start=(k == 0), stop=(k == KD - 1))
                    nc.vector.scalar_tensor_tensor(
                        W(0, 8, c * TC, (c + 1) * TC),
                        nz[:, c * TC:(c + 1) * TC], NOISE_STD, lp[:],
                        op0=ALU.mult, op1=ALU.add)

                # (max, 2nd-max) tournament across the 8 expert rows
                tt = nc.vector.tensor_tensor
                tt(W(8, 12), W(0, 4), W(4, 8), op=ALU.max)      # R1X
                tt(W(12, 16), W(0, 4), W(4, 8), op=ALU.min)     # R1N
                tt(W(16, 18), W(8, 10), W(10, 12), op=ALU.max)  # R2X
                tt(W(18, 20), W(8, 10), W(10, 12), op=ALU.min)  # R2N
                tt(W(20, 22), W(12, 14), W(14, 16), op=ALU.max)  # R2B
                tt(W(22, 24), W(18, 20), W(20, 22), op=ALU.max)  # R2S
                tt(W(24, 25), W(16, 17), W(17, 18), op=ALU.max)  # M1
                tt(W(25, 26), W(16, 17), W(17, 18), op=ALU.min)  # R3N
                tt(W(26, 27), W(22, 23), W(23, 24), op=ALU.max)  # R3B
                tt(W(27, 28), W(25, 26), W(26, 27), op=ALU.max)  # M2
                tt(W(28, 29), W(24, 25), W(27, 28), op=ALU.add)  # M12
                # exact partition broadcasts on the (idle) GPSIMD engine
                nc.gpsimd.partition_broadcast(W(48, 56), W(28, 29))  # m1+m2
                nc.gpsimd.partition_broadcast(W(56, 64), W(27, 28))  # m2
                # q = 2*noisy - (m1+m2);  mask = noisy >= m2 (bit-exact)
                nc.vector.scalar_tensor_tensor(
                    W(32, 40), W(0, 8), 2.0, W(48, 56),
                    op0=ALU.mult, op1=ALU.subtract)
                tt(gbm[8:16, :], W(0, 8), W(56, 64), op=ALU.is_ge)

                def emit_gate_tail():
                    # after chunk-0 B1 gelus: one Sigmoid table swap per iter
                    nc.scalar.activation(gbm[0:8, :], W(32, 40), AF.Sigmoid)
                    nc.vector.tensor_tensor(wt[:], gbm[0:8, :], gbm[8:16, :],
                                            op=ALU.mult)

                # ---------- chunks ----------
                for c in range(NCH):
                    c0 = c * TC
                    x_t = x_cur
                    x_nxt = load_x(c + 1) if c + 1 < NCH else None

                    ht = []
                    for km in range(KM):
                        hp = ps_h.tile([128, TC], F32, tag='hT',
                                       name=f'hp{km}_{c}_{it}')
                        for k in range(KD):
                            nc.tensor.matmul(
                                hp[:], w1_t[k][:, km * 128:(km + 1) * 128],
                                x_t[k][:], start=(k == 0), stop=(k == KD - 1))
                        hg = htp.tile([128, TC + ht_pad], BF16, tag='ht',
                                      name=f'ht{km}_{c}_{it}')
                        nc.scalar.activation(hg[:, :TC], hp[:],
                                             AF.Gelu_apprx_tanh)
                        ht.append(hg)

                    if c == 0:
                        emit_gate_tail()

                    # experts
                    hs = []
                    for half in range(2):
                        hp2 = ps_a.tile([96, TC], F32, tag='mA',
                                        name=f'hc{half}_{c}_{it}')
                        for k in range(KD):
                            nc.tensor.matmul(
                                hp2[:],
                                we1_t[:, k * EI + half * 96:
                                      k * EI + (half + 1) * 96],
                                x_t[k][:], start=(k == 0), stop=(k == KD - 1))
                        hg2 = ep.tile([96, TC], BF16, tag=f'hg{half}',
                                      name=f'hg{half}_{c}_{it}')
                        nc.scalar.activation(hg2[:], hp2[:], AF.Gelu_apprx_tanh)
                        wp = ps_a.tile([96, TC], F32, tag='mA',
                                       name=f'wp{half}_{c}_{it}')
                        nc.tensor.matmul(
                            wp[:], bb_t[:, half * 96:(half + 1) * 96],
                            wt[:, c0:c0 + TC], start=True, stop=True)
                        hsc = ep.tile([96, TC], BF16, tag=f'hs{half}',
                                      name=f'hs{half}_{c}_{it}')
                        nc.vector.tensor_tensor(hsc[:], hg2[:], wp[:],
                                                op=ALU.mult)
                        hs.append(hsc)

                    emit_b2(c, ht, hs, it)

                    x_cur = x_nxt

            if phases != 'full':
                body = emit_body_iso
            elif gate_mode == 'sig':
                body = emit_body_sig
            else:
                body = emit_body
            if loop_reps is None:
                body(0)
            else:
                with tc.For_i(0, loop_reps, 1,
                              hint_engines=(mybir.EngineType.PE,)) as _:
                    body(0)

    nc.compile()
    return nc


_nc_cache = [None]


def _prep_host(inputs):
    inputs = {k: np.asarray(v) for k, v in inputs.items()}
    x = np.ascontiguousarray(inputs['x'], np.float32).reshape(B * N, D)
    noise = np.ascontiguousarray(inputs['noise'], np.float32).reshape(B * N, E)
    W1eff = (inputs['W1'] + inputs['dW1']).astype(np.float32)   # [MID, D]
    W2eff = (inputs['W2'] + inputs['dW2']).astype(np.float32)   # [D, MID]
    w1T = np.ascontiguousarray(
        W1eff.T.reshape(KD, 128, MID)).astype(NP_BF16)
    w2T = np.ascontiguousarray(
        W2eff.T.reshape(KM, 128, D)).astype(NP_BF16)
    wgT = np.ascontiguousarray(np.asarray(inputs['Wg'], np.float32).T)  # [D, E]
    We1 = np.asarray(inputs['We1'], np.float32)                  # [E, INNER, D]
    We2 = np.asarray(inputs['We2'], np.float32)                  # [E, D, INNER]
    we1T = np.ascontiguousarray(We1.reshape(EI, D).T).astype(NP_BF16)
    we2T = np.ascontiguousarray(
        We2.transpose(0, 2, 1).reshape(EI, D)).astype(NP_BF16)
    bb = np.zeros((E, EI), np.float32)
    for e in range(E):
        bb[e, e * INNER:(e + 1) * INNER] = 1.0
    bb = bb.astype(NP_BF16)
    xT = np.ascontiguousarray(x.T)                               # [D, B*N] f32
    xT_bf = xT.astype(NP_BF16)
    return xT, xT_bf, noise, w1T, w2T, wgT, we1T, we2T, bb


def _make_in_maps(hosts):
    xT, xT_bf, noise, w1T, w2T, wgT, we1T, we2T, bb = hosts
    in_maps = []
    for c in range(NCORES):
        t0 = c * TPC
        xc = np.ascontiguousarray(
            np.ascontiguousarray(xT_bf[:, t0:t0 + TPC])
            .reshape(KD, 128, NCH, TC).transpose(2, 0, 1, 3))
        xgc = np.ascontiguousarray(
            np.ascontiguousarray(xT[:, t0:t0 + TPC])
            .reshape(KD, 128, NCH, TC).transpose(2, 0, 1, 3))
        in_maps.append({
            'xT': xc,
            'xgT': xgc,
            'noise': np.ascontiguousarray(noise[t0:t0 + TPC, :]),
            'noiseT': np.ascontiguousarray(noise[t0:t0 + TPC, :].T),
            'w1T': w1T, 'w2T': w2T, 'wgT': wgT,
            'we1T': we1T, 'we2T': we2T, 'bb': bb,
        })
    return in_maps


def kernel(**inputs):
    hosts = _prep_host(inputs)
    if _nc_cache[0] is None:
        _nc_cache[0] = build_core_program()
    nc = _nc_cache[0]

    in_maps = _make_in_maps(hosts)

    from concourse.bass_utils import run_bass_kernel_spmd
    res = run_bass_kernel_spmd(nc, in_maps, core_ids=list(range(NCORES)))
    out = np.empty((B * N, D), np.float32)
    for c in range(NCORES):
        out[c * TPC:(c + 1) * TPC, :] = res.results[c]['yT'].T
    return out.reshape(B, N, D)

